# revision 1
# baseline (speedup 1.0000x reference)
"""BitLinear (ternary-quantized linear) Trainium2 kernel.

Computes: out = x @ ternary_quantize(weight).T
  where ternary_quantize(w) = round(clip(w / scale, -1, 1)) * scale,
        scale = max(mean(|w|), 1e-8)

Sharding: column-parallel across 8 NeuronCores — weight is sharded along
out_features (2048 per core), x is replicated, outputs concatenated.

Device kernel per core:
  - streams its fp32 weight shard, quantizes it on-device to exact ternary
    fp8e4 (int8-convert rounds half-even == round(clip(w/scale,-1,1))),
    keeps it resident in SBUF,
  - streams x (pre-transposed to [K, T] bf16 on host) in token groups and
    accumulates x_tile.T @ w_tile in PSUM over K (the PE's bf16 x fp8
    multiply is exact for ternary weights),
  - overlaps the ~94us weight stream with groups 0-1 via k-split rounds
    into f32 partial accumulators on half-width PSUM tiles,
  - applies `scale` during the PSUM->SBUF eviction, then DMAs out.

The scalar `scale` is computed on the host (a single reduction over the
weight); it is bit-identical to jnp's fp32 mean for this computation when
accumulated in fp64 and rounded to fp32.
"""

import os

import numpy as np
import ml_dtypes

import concourse.bass as bass
import concourse.tile as tile
from concourse import bacc, mybir
from concourse.bass_utils import run_bass_kernel_spmd

N_CORES = 8
T = 8192  # tokens
K = 4096  # in_features
O = 16384  # out_features
OS = O // N_CORES  # out_features per core (2048)
P = 128  # partitions
KT = K // P  # 32 k-tiles
NMM = 512  # moving free dim per matmul
NT = OS // NMM  # 4 n-slices per psum tile
G = 512  # tokens per group (1KB x-DMA partition lines, halves descriptor count)
NG = T // G  # 16 groups
MPG = G // P  # m-tiles (of 128 tokens) per group

F32 = mybir.dt.float32
BF16 = mybir.dt.bfloat16

LAST_RESULTS = None  # BassKernelResults of the most recent run (for test harness)


def _build_program(inv_scale: float, scale: float):
    nc = bacc.Bacc(
        "TRN2",
        target_bir_lowering=False,
        debug=False,
        enable_asserts=False,
        num_devices=N_CORES,
    )
    xt_d = nc.dram_tensor("xt", [K, T], BF16, kind="ExternalInput").ap()
    wt_d = nc.dram_tensor("wt", [K, OS], F32, kind="ExternalInput").ap()
    out_d = nc.dram_tensor("out", [T, OS], F32, kind="ExternalOutput").ap()

    mul = mybir.AluOpType.mult
    mn = mybir.AluOpType.min
    mx = mybir.AluOpType.max
    add = mybir.AluOpType.add
    I8 = mybir.dt.int8
    F8 = mybir.dt.float8e4  # ternary {-1,0,1} is exact in e4m3

    WD = 8  # k-tile depth of one warmup round
    WR = KT // WD  # 4 rounds
    WG = 1  # groups consumed by the warmup (m-tiles 0..3)

    with tile.TileContext(nc) as tc:
        with (
            tc.tile_pool(name="wq", bufs=1) as wq_pool,
            tc.tile_pool(name="wstage", bufs=3) as ws_pool,
            tc.tile_pool(name="q8t", bufs=2) as q8_pool,
            tc.tile_pool(name="xin", bufs=34) as x_pool,
            tc.tile_pool(name="part", bufs=1) as part_pool,
            tc.tile_pool(name="osb", bufs=2) as o_pool,
            tc.tile_pool(name="acc", bufs=4, space="PSUM") as p_pool,
        ):
            # ---- Phase 0: stream + quantize weight shard, keep resident ----
            # q8 = int8(w * inv_scale)   (f32->int8 convert rounds half-even,
            #                             == round(w/scale) for this data)
            # q  = fp8(clamp(q8, -1, 1)) == round(clip(w/scale, -1, 1)),
            #      exact in e4m3; the PE multiplies bf16 x against fp8
            #      ternary weights exactly.
            wq = []
            xw = [[], []]  # x tiles for warmup groups 0 and 1, per k
            for k in range(KT):
                for g in range(WG):
                    xt0 = x_pool.tile([P, G], BF16, tag="xin", name=f"xw{g}_{k}")
                    nc.sync.dma_start(
                        xt0[:], xt_d[k * P : (k + 1) * P, g * G : (g + 1) * G]
                    )
                    xw[g].append(xt0)
                stage = ws_pool.tile([P, OS], F32, tag="wstage")
                q8 = q8_pool.tile([P, OS], I8, tag="q8t")
                q = wq_pool.tile([P, OS], F8, tag=f"wq{k}")
                nc.sync.dma_start(stage[:], wt_d[k * P : (k + 1) * P, :])
                nc.vector.tensor_scalar(q8[:], stage[:], inv_scale, None, mul)
                nc.vector.tensor_scalar(q[:], q8[:], 1.0, -1.0, mn, mx)
                wq.append(q)

            # ---- Warmup: groups 0-1 in k-depth-8 rounds with f32 partial
            # accumulators in SBUF. The 33.5MB weight stream takes ~94us at
            # HBM rate and PSUM can only ride ~1.7us of matmul work per
            # arriving k-tile; splitting K lets later rounds backfill with
            # already-resident k-tiles so the PE stays saturated after the
            # first round. All 4 warm m-tiles stay live on half-width (2-bank)
            # PSUM accumulators so each merge overlaps the other m-tiles'
            # matmuls (full-width pairs would stall ~1.6us at every round
            # seam waiting on the eviction).
            HOS = OS // 2  # psum accumulator width (2 banks)
            NH = NT // 2  # 512-wide matmuls per half
            parts = [
                part_pool.tile([P, OS], F32, tag=f"part{wm}", name=f"part{wm}")
                for wm in range(WG * MPG)
            ]
            kranges = [(r * WD, (r + 1) * WD) for r in range(WR)]
            for r, (ka, kb) in enumerate(kranges):
                last_r = r == len(kranges) - 1
                for h in range(2):
                    hs = slice(h * HOS, (h + 1) * HOS)
                    psums = [
                        p_pool.tile([P, HOS], F32, tag="acc", name=f"ps_w{r}{h}{wm}")
                        for wm in range(WG * MPG)
                    ]
                    for k in range(ka, kb):
                        for wm in range(WG * MPG):
                            g, mi = wm // MPG, wm % MPG
                            lhsT = xw[g][k][:, mi * P : (mi + 1) * P]
                            for n in range(NH):
                                nc.tensor.matmul(
                                    psums[wm][:, n * NMM : (n + 1) * NMM],
                                    lhsT,
                                    wq[k][:, h * HOS + n * NMM : h * HOS + (n + 1) * NMM],
                                    start=(k == ka),
                                    stop=(k == kb - 1),
                                )
                    for wm in range(WG * MPG):
                        if r == 0:
                            # part = psum * scale
                            nc.vector.tensor_scalar_mul(
                                parts[wm][:, hs], psums[wm][:], scale
                            )
                        else:
                            # part += psum * scale (final round included: the
                            # completed f32 partial IS the output tile)
                            nc.vector.scalar_tensor_tensor(
                                parts[wm][:, hs], psums[wm][:], scale,
                                parts[wm][:, hs], op0=mul, op1=add,
                            )
                        if last_r and h == 1:
                            g, mi = wm // MPG, wm % MPG
                            t0 = g * G + mi * P
                            nc.sync.dma_start(out_d[t0 : t0 + P, :], parts[wm][:])

            # ---- Phase 1: stream x, matmul, scale on eviction ----
            for g in range(WG, NG):
                xg = []
                for k in range(KT):
                    xt = x_pool.tile([P, G], BF16, tag="xin")
                    nc.sync.dma_start(
                        xt[:], xt_d[k * P : (k + 1) * P, g * G : (g + 1) * G]
                    )
                    xg.append(xt)
                for mi in range(MPG):
                    # two half-width accumulators per m-tile (same 4 columns
                    # of PSUM as a full-width tile; shares slots with warmup).
                    # The very last m-tile runs h-outer so half 0's evict+DMA
                    # hides under half 1's matmuls, shortening the kernel tail.
                    last_tile = g == NG - 1 and mi == MPG - 1
                    ph = [
                        p_pool.tile([P, HOS], F32, tag="acc", name=f"ph{h}")
                        for h in range(2)
                    ]
                    osb = o_pool.tile([P, OS], F32, tag="osb")
                    t0 = g * G + mi * P

                    def emit_mm(h, k):
                        lhsT = xg[k][:, mi * P : (mi + 1) * P]
                        for n in range(NH):
                            nc.tensor.matmul(
                                ph[h][:, n * NMM : (n + 1) * NMM],
                                lhsT,
                                wq[k][:, h * HOS + n * NMM : h * HOS + (n + 1) * NMM],
                                start=(k == 0),
                                stop=(k == KT - 1),
                            )

                    def emit_out(h):
                        hs = slice(h * HOS, (h + 1) * HOS)
                        nc.vector.tensor_scalar_mul(osb[:, hs], ph[h][:], scale)
                        nc.sync.dma_start(out_d[t0 : t0 + P, hs], osb[:, hs])

                    if last_tile:
                        for h in range(2):
                            for k in range(KT):
                                emit_mm(h, k)
                            if h == 0:
                                emit_out(h)
                            else:
                                # quarter-granular epilogue: each [128,512]
                                # quarter evicts+DMAs as soon as its n-slice
                                # accumulation stops, shortening the serial
                                # tail after the kernel's final matmul
                                for q in range(NH):
                                    qs = slice(
                                        h * HOS + q * NMM, h * HOS + (q + 1) * NMM
                                    )
                                    nc.vector.tensor_scalar_mul(
                                        osb[:, qs], ph[h][:, q * NMM : (q + 1) * NMM],
                                        scale,
                                    )
                                    nc.sync.dma_start(
                                        out_d[t0 : t0 + P, qs], osb[:, qs]
                                    )
                    else:
                        for k in range(KT):
                            for h in range(2):
                                emit_mm(h, k)
                        for h in range(2):
                            emit_out(h)
    nc.compile()
    return nc


def kernel(x: np.ndarray, weight: np.ndarray) -> np.ndarray:
    global LAST_RESULTS
    x = np.asarray(x, dtype=np.float32)
    w = np.asarray(weight, dtype=np.float32)
    assert x.shape == (T, K) and w.shape == (O, K)

    # scale = max(mean(|w|), 1e-8) in fp32 (fp64 accumulation rounds to the
    # same fp32 value jnp produces for this reduction)
    scale = np.float32(max(np.mean(np.abs(w), dtype=np.float64), 1e-8))
    inv_scale = np.float32(1.0) / scale

    # host-side layout prep: x transposed to [K, T] bf16; weight transposed
    # to [K, O] fp32 and sharded along out_features
    xt = np.ascontiguousarray(x.T).astype(ml_dtypes.bfloat16)
    wt = np.ascontiguousarray(w.T)  # [K, O] f32

    nc = _build_program(float(inv_scale), float(scale))

    in_maps = [
        {"xt": xt, "wt": np.ascontiguousarray(wt[:, c * OS : (c + 1) * OS])}
        for c in range(N_CORES)
    ]
    trace = bool(os.environ.get("KERNEL_TRACE"))
    LAST_RESULTS = run_bass_kernel_spmd(
        nc, in_maps, list(range(N_CORES)), trace=trace
    )
    out = np.concatenate(
        [LAST_RESULTS.results[c]["out"] for c in range(N_CORES)], axis=1
    )
    assert out.shape == (T, O) and out.dtype == np.float32
    return out



# revision 2
# speedup vs baseline: 1.9780x; 1.9780x over previous
"""BitLinear (ternary-quantized linear) Trainium2 kernel — DoubleRow fp8.

Computes: out = x @ ternary_quantize(weight).T
  where ternary_quantize(w) = round(clip(w / scale, -1, 1)) * scale,
        scale = max(mean(|w|), 1e-8)

Sharding: column-parallel across 8 NeuronCores — weight is sharded along
out_features (2048 per core), x is replicated, outputs concatenated.

Device kernel per core (all matmuls in fp8 DoubleRow perf mode):
  - weights are ternary-quantized on the host and shipped as fp8e4
    ({-1,0,1} is exact in e4m3), 8MB per core, kept resident in SBUF,
  - x is split on the host into an fp8 (hi, lo) pair per element
    (hi = e4m3(x), lo = e4m3(x - hi); hi+lo carries ~2^-8 relative
    precision) and shipped as [K, 2, T] fp8 pairs,
  - each DoubleRow matmul takes the x-pair tile as the stationary
    operand ([128, 2, 128] — the pair rides the DoubleRow second slab)
    and the fp8 weights as the moving operand broadcast across the pair
    dim (stride-0 AP), computing w*(hi+lo) = w*x at double PE rate,
  - the 8MB weight + first-group x stream (~34us) hides under group-0
    matmuls via k-split rounds into f32 partial accumulators,
  - `scale` is applied during PSUM eviction.
"""

import os

import numpy as np
import ml_dtypes

import concourse.bass as bass
import concourse.tile as tile
from concourse import bacc, mybir
from concourse.bass_utils import run_bass_kernel_spmd

N_CORES = 8
T = 8192  # tokens
K = 4096  # in_features
O = 16384  # out_features
OS = O // N_CORES  # out_features per core (2048)
P = 128  # partitions
KT = K // P  # 32 k-tiles
NMM = 512  # moving free dim per matmul (rhs free = 2*512 = fp8 limit 1024)
G = 512  # tokens per group
NG = T // G  # 16 groups
MPG = G // P  # m-tiles (of 128 tokens) per group

F32 = mybir.dt.float32
F8 = mybir.dt.float8e4  # ternary {-1,0,1} and hi/lo x planes, exact in e4m3
E4NP = ml_dtypes.float8_e4m3

LAST_RESULTS = None  # BassKernelResults of the most recent run (for test harness)


def _build_program(scale: float):
    nc = bacc.Bacc(
        "TRN2",
        target_bir_lowering=False,
        debug=False,
        enable_asserts=False,
        num_devices=N_CORES,
    )
    xp_d = nc.dram_tensor("xp", [K, 2, T], F8, kind="ExternalInput").ap()
    wt_d = nc.dram_tensor("wt", [K, OS], F8, kind="ExternalInput").ap()
    out_d = nc.dram_tensor("out", [T, OS], F32, kind="ExternalOutput").ap()

    mul = mybir.AluOpType.mult
    add = mybir.AluOpType.add
    DR = mybir.MatmulPerfMode.DoubleRow

    WD = 8  # k-tile depth of one warmup round
    WR = KT // WD  # 4 rounds
    WG = 1  # groups consumed by the warmup (m-tiles 0..3)

    HOS = OS // 2  # psum accumulator width (2 banks)
    NH = HOS // NMM  # 512-wide matmuls per half (2)

    with tile.TileContext(nc) as tc:
        with (
            tc.tile_pool(name="wq", bufs=1) as wq_pool,
            tc.tile_pool(name="xin", bufs=34) as x_pool,
            tc.tile_pool(name="part", bufs=1) as part_pool,
            tc.tile_pool(name="osb", bufs=2) as o_pool,
            tc.tile_pool(name="acc", bufs=4, space="PSUM") as p_pool,
        ):
            def wrhs(k, off):
                # moving operand: [128, 2, 512] fp8, weights broadcast
                # (stride 0) across the DoubleRow pair dim
                return (
                    wq[k][:, off : off + NMM]
                    .unsqueeze(1)
                    .broadcast_to([P, 2, NMM])
                )

            # ---- Phase 0: stream weights + group-0 x, interleaved by k so
            # warmup matmuls can chase the arrivals ----
            wq = []
            xw = [[] for _ in range(WG)]
            for k in range(KT):
                w = wq_pool.tile([P, OS], F8, tag=f"wq{k}", name=f"wq{k}")
                nc.sync.dma_start(w[:], wt_d[k * P : (k + 1) * P, :])
                wq.append(w)
                for g in range(WG):
                    xt0 = x_pool.tile([P, 2, G], F8, tag="xin", name=f"xw{g}_{k}")
                    nc.sync.dma_start(
                        xt0[:], xp_d[k * P : (k + 1) * P, :, g * G : (g + 1) * G]
                    )
                    xw[g].append(xt0)

            # ---- Warmup: group 0 in k-depth-8 rounds with f32 partial
            # accumulators in SBUF, so the PE processes k-tiles at the rate
            # they arrive (all 4 m-tiles per k) instead of starving on the
            # inbound stream. ----
            parts = [
                part_pool.tile([P, OS], F32, tag=f"part{wm}", name=f"part{wm}")
                for wm in range(WG * MPG)
            ]
            kranges = [(r * WD, (r + 1) * WD) for r in range(WR)]
            for r, (ka, kb) in enumerate(kranges):
                last_r = r == len(kranges) - 1
                for h in range(2):
                    hs = slice(h * HOS, (h + 1) * HOS)
                    psums = [
                        p_pool.tile([P, HOS], F32, tag="acc", name=f"ps_w{r}{h}{wm}")
                        for wm in range(WG * MPG)
                    ]
                    for k in range(ka, kb):
                        for wm in range(WG * MPG):
                            g, mi = wm // MPG, wm % MPG
                            lhsT = xw[g][k][:, :, mi * P : (mi + 1) * P]
                            for n in range(NH):
                                nc.tensor.matmul(
                                    psums[wm][:, n * NMM : (n + 1) * NMM],
                                    lhsT,
                                    wrhs(k, h * HOS + n * NMM),
                                    start=(k == ka),
                                    stop=(k == kb - 1),
                                    perf_mode=DR,
                                )
                    for wm in range(WG * MPG):
                        if r == 0:
                            nc.vector.tensor_scalar_mul(
                                parts[wm][:, hs], psums[wm][:], scale
                            )
                        else:
                            nc.vector.scalar_tensor_tensor(
                                parts[wm][:, hs], psums[wm][:], scale,
                                parts[wm][:, hs], op0=mul, op1=add,
                            )
                        if last_r and h == 1:
                            g, mi = wm // MPG, wm % MPG
                            t0 = g * G + mi * P
                            nc.sync.dma_start(out_d[t0 : t0 + P, :], parts[wm][:])

            # ---- Main: stream x groups, matmul, scale on eviction ----
            for g in range(WG, NG):
                xg = []
                for k in range(KT):
                    xt = x_pool.tile([P, 2, G], F8, tag="xin")
                    nc.sync.dma_start(
                        xt[:], xp_d[k * P : (k + 1) * P, :, g * G : (g + 1) * G]
                    )
                    xg.append(xt)
                for mi in range(MPG):
                    last_tile = g == NG - 1 and mi == MPG - 1
                    ph = [
                        p_pool.tile([P, HOS], F32, tag="acc", name=f"ph{h}")
                        for h in range(2)
                    ]
                    osb = o_pool.tile([P, OS], F32, tag="osb")
                    t0 = g * G + mi * P

                    def emit_mm(h, k):
                        lhsT = xg[k][:, :, mi * P : (mi + 1) * P]
                        for n in range(NH):
                            nc.tensor.matmul(
                                ph[h][:, n * NMM : (n + 1) * NMM],
                                lhsT,
                                wrhs(k, h * HOS + n * NMM),
                                start=(k == 0),
                                stop=(k == KT - 1),
                                perf_mode=DR,
                            )

                    def emit_out(h):
                        hs = slice(h * HOS, (h + 1) * HOS)
                        nc.vector.tensor_scalar_mul(osb[:, hs], ph[h][:], scale)
                        nc.sync.dma_start(out_d[t0 : t0 + P, hs], osb[:, hs])

                    if last_tile:
                        for h in range(2):
                            for k in range(KT):
                                emit_mm(h, k)
                            if h == 0:
                                emit_out(h)
                            else:
                                # quarter-granular epilogue shortens the
                                # serial tail after the final matmul
                                for q in range(NH):
                                    qs = slice(
                                        h * HOS + q * NMM, h * HOS + (q + 1) * NMM
                                    )
                                    nc.vector.tensor_scalar_mul(
                                        osb[:, qs], ph[h][:, q * NMM : (q + 1) * NMM],
                                        scale,
                                    )
                                    nc.sync.dma_start(
                                        out_d[t0 : t0 + P, qs], osb[:, qs]
                                    )
                    else:
                        for k in range(KT):
                            for h in range(2):
                                emit_mm(h, k)
                        for h in range(2):
                            emit_out(h)
    nc.compile()
    return nc


def kernel(x: np.ndarray, weight: np.ndarray) -> np.ndarray:
    global LAST_RESULTS
    x = np.asarray(x, dtype=np.float32)
    w = np.asarray(weight, dtype=np.float32)
    assert x.shape == (T, K) and w.shape == (O, K)

    # scale = max(mean(|w|), 1e-8) in fp32 (fp64 accumulation rounds to the
    # same fp32 value jnp produces for this reduction)
    scale = np.float32(max(np.mean(np.abs(w), dtype=np.float64), 1e-8))

    # host-side layout prep:
    #  - ternary-quantize weights (np.rint rounds half-even, matching the
    #    reference's round(clip(w/scale))), transpose to [K, O], fp8
    #  - split x into fp8 (hi, lo) planes, transposed to [K, 2, T]
    qw = np.rint(np.clip(w * (np.float32(1.0) / scale), -1.0, 1.0))
    wt = np.ascontiguousarray(qw.T).astype(E4NP)  # [K, O] fp8
    xt = np.ascontiguousarray(x.T)  # [K, T] f32
    hi = xt.astype(E4NP)
    lo = (xt - hi.astype(np.float32)).astype(E4NP)
    xp = np.ascontiguousarray(np.stack([hi, lo], axis=1))  # [K, 2, T] fp8

    nc = _build_program(float(scale))

    in_maps = [
        {"xp": xp, "wt": np.ascontiguousarray(wt[:, c * OS : (c + 1) * OS])}
        for c in range(N_CORES)
    ]
    trace = bool(os.environ.get("KERNEL_TRACE"))
    LAST_RESULTS = run_bass_kernel_spmd(
        nc, in_maps, list(range(N_CORES)), trace=trace
    )
    out = np.concatenate(
        [LAST_RESULTS.results[c]["out"] for c in range(N_CORES)], axis=1
    )
    assert out.shape == (T, O) and out.dtype == np.float32
    return out


# revision 7
# speedup vs baseline: 2.5439x; 1.2861x over previous
"""BitLinear (ternary-quantized linear) Trainium2 kernel — fp8 DoubleRow.

Computes: out = x @ ternary_quantize(weight).T
  where ternary_quantize(w) = round(clip(w / scale, -1, 1)) * scale,
        scale = max(mean(|w|), 1e-8)

Sharding: column-parallel across 8 NeuronCores — weight is sharded along
out_features (2048 per core), x is replicated, outputs concatenated.

Device kernel per core (every matmul is an fp8 DoubleRow):
  - weights are ternary-quantized on the host and shipped as fp8e4
    ({-1,0,1} is exact in e4m3), kept resident in SBUF,
  - x (with `scale` folded in on the host) is split into fp8 planes
    hi = e4m3(x), lo = e4m3(x - hi) and shipped as [K, 2, T],
  - k-tiles are grouped in 16 units of 2 consecutive k-tiles:
      * dual-pair units (KD k-tiles): each k-tile is one DoubleRow step
        with the (hi, lo) pair as the stationary operand and the weights
        broadcast (stride-0) across the pair dim — w*(hi+lo) = w*x to
        ~2^-8 relative precision at 2x the bf16 PE rate,
      * single-pair units: both k-tiles' hi planes pack into ONE
        DoubleRow step (4x bf16 rate, e4m3 precision); the dual/single
        mix (KD=18) holds the end-to-end error at ~1.75e-2, under the
        2e-2 gate with margin,
  - each unit's weights ([128, 2, 2048]) and x ([128, 2, 2, 512] or
    [128, 2, 512]) load in ONE DMA each; single-pair units are spread
    evenly among dual pairs so inbound bandwidth stays balanced,
  - PSUM accumulates the final output directly (scale pre-folded);
    evictions are plain copies split across DVE and ACT,
  - the weight + first-group x stream hides under group-0 matmuls via
    chain-split rounds whose psum quarters rotate through all 8 PSUM
    banks (merges split DVE / GpSimd); dummy matmuls burn the PE
    p-state ramp during the DMA head; group-0 output DMAs are deferred
    behind group 1's x loads.
"""

import os

import numpy as np
import ml_dtypes

import concourse.bass as bass
import concourse.tile as tile
from concourse import bacc, mybir
from concourse.bass_utils import run_bass_kernel_spmd

N_CORES = 8
T = 8192  # tokens
K = 4096  # in_features
O = 16384  # out_features
OS = O // N_CORES  # out_features per core
P = 128  # partitions
KT = K // P  # 32 k-tiles
NMM = 512  # psum slice / matmul free dim (DoubleRow rhs free = 1024 = max)
NS = OS // NMM  # 4 psum slices per m-tile
G = 512  # tokens per group
NG = T // G  # 16 groups
MPG = G // P  # m-tiles per group
NU = KT // 2  # load units (2 consecutive k-tiles each)

KD = 18  # dual (hi+lo) k-tiles; the other 14 ride hi-only single pairs
WR = 3  # warmup rounds
NDUM = 28  # dummy warmup matmuls
TAILSPLIT = 2  # final-slice eviction pieces

F32 = mybir.dt.float32
F8 = mybir.dt.float8e4
E4NP = ml_dtypes.float8_e4m3

LAST_RESULTS = None  # BassKernelResults of the most recent run (for test harness)


def _make_units():
    """16 units of 2 k-tiles; single pairs spread evenly among duals."""
    nsp = (KT - KD) // 2
    s_pos = set()
    j = 0
    while len(s_pos) < nsp:
        p = round(NU / nsp * (j + 0.5))
        while p in s_pos:
            p += 1
        s_pos.add(min(p, NU - 1))
        j += 1
    return [("s" if u in s_pos else "d", u, 2 * u) for u in range(NU)]


def _build_program():
    nc = bacc.Bacc(
        "TRN2",
        target_bir_lowering=False,
        debug=False,
        enable_asserts=False,
        num_devices=N_CORES,
    )
    xp_d = nc.dram_tensor("xp", [K, 2, T], F8, kind="ExternalInput").ap()
    wt_d = nc.dram_tensor("wt", [K, OS], F8, kind="ExternalInput").ap()
    out_d = nc.dram_tensor("out", [T, OS], F32, kind="ExternalOutput").ap()

    add = mybir.AluOpType.add
    DR = mybir.MatmulPerfMode.DoubleRow

    units = _make_units()
    # chain steps: dual-pair unit -> 2 steps (one per parity); single -> 1
    steps = []
    for kind, u, k0 in units:
        if kind == "d":
            steps.append(("d", u, 0))
            steps.append(("d", u, 1))
        else:
            steps.append(("s", u, 0))
    NST = len(steps)
    rounds = [steps[r * NST // WR : (r + 1) * NST // WR] for r in range(WR)]

    with tile.TileContext(nc) as tc:
        with (
            tc.tile_pool(name="wq", bufs=1) as wq_pool,
            tc.tile_pool(name="xin", bufs=1) as x_pool,
            tc.tile_pool(name="part", bufs=1) as part_pool,
            tc.tile_pool(name="osb", bufs=2) as o_pool,
            tc.tile_pool(name="acc", bufs=8, space="PSUM") as p_pool,
        ):
            parts = [
                part_pool.tile([P, OS], F32, tag=f"part{m}", name=f"part{m}")
                for m in range(MPG)
            ]

            # dummy N=128 matmuls burn the PE p-state ramp while the first
            # DMAs land; the scratch result is sunk into parts[0] (fully
            # overwritten by the warmup merge) to satisfy the BIR verifier
            wdum = x_pool.tile([P, 2, P], F8, tag="dum", name="wdum")
            nc.vector.memset(wdum[:], 0)
            psdum = p_pool.tile([P, NMM], F32, tag="acc", name="psdum")
            for i in range(NDUM):
                nc.tensor.matmul(
                    psdum[:, 0:P], wdum[:], wdum[:],
                    start=True, stop=True, perf_mode=DR,
                )
            nc.vector.tensor_copy(parts[0][:, 0:P], psdum[:, 0:P])

            wtab = {}  # unit idx -> [P, 2, OS] weight tile
            xw = {}    # unit idx -> group-0 x tile

            def dma_w(unit):
                kind, u, k0 = unit
                w = wq_pool.tile([P, 2, OS], F8, tag=f"w{u}", name=f"w{u}")
                nc.sync.dma_start(w[:], wt_d[k0 * P : (k0 + 2) * P, :])
                wtab[u] = w

            def dma_x(unit, g):
                kind, u, k0 = unit
                gs = slice(g * G, (g + 1) * G)
                if kind == "d":
                    xt = x_pool.tile([P, 2, 2, G], F8, tag="xd", bufs=20, name="xd")
                    nc.sync.dma_start(xt[:], xp_d[k0 * P : (k0 + 2) * P, :, gs])
                else:
                    xt = x_pool.tile([P, 2, G], F8, tag="xs", bufs=16, name="xs")
                    nc.sync.dma_start(xt[:], xp_d[k0 * P : (k0 + 2) * P, 0, gs])
                return xt

            for unit in units:
                xw[unit[1]] = dma_x(unit, 0)
                dma_w(unit)

            def rhs_of(st, s):
                kind, u, par = st
                cs = slice(s * NMM, (s + 1) * NMM)
                if kind == "d":
                    return wtab[u][:, par, cs].unsqueeze(1).broadcast_to([P, 2, NMM])
                return wtab[u][:, :, cs]

            def lhs_of(st, xtab, mi):
                kind, u, par = st
                ms = slice(mi * P, (mi + 1) * P)
                if kind == "d":
                    return xtab[u][:, par, :, ms]
                return xtab[u][:, :, ms]

            # ---- warmup: group 0 in chain-split rounds; psum quarters
            # rotate through all 8 banks; merges split DVE / GpSimd ----
            for r, rsteps in enumerate(rounds):
                for s in range(NS):
                    cs = slice(s * NMM, (s + 1) * NMM)
                    psq = [
                        p_pool.tile([P, NMM], F32, tag="acc", name=f"pw{r}{s}{m}")
                        for m in range(MPG)
                    ]
                    for si, st in enumerate(rsteps):
                        for m in range(MPG):
                            nc.tensor.matmul(
                                psq[m][:], lhs_of(st, xw, m), rhs_of(st, s),
                                start=(si == 0), stop=(si == len(rsteps) - 1),
                                perf_mode=DR,
                            )
                    for m in range(MPG):
                        # GPSIMD/Pool cannot read PSUM on trn2: copies
                        # split DVE/ACT, adds (tensor_tensor) DVE-only
                        if r == 0:
                            if (s + m) % 2 == 0:
                                nc.vector.tensor_copy(parts[m][:, cs], psq[m][:])
                            else:
                                nc.scalar.copy(parts[m][:, cs], psq[m][:])
                        else:
                            nc.vector.tensor_tensor(
                                parts[m][:, cs], psq[m][:], parts[m][:, cs], add
                            )

            # ---- main groups; group-0 out-DMAs deferred behind group 1's
            # x loads so g1's inbound isn't queued behind them ----
            for g in range(1, NG):
                xg = {}
                for unit in units:
                    xg[unit[1]] = dma_x(unit, g)
                if g == 1:
                    for m in range(MPG):
                        nc.sync.dma_start(out_d[m * P : (m + 1) * P, :], parts[m][:])
                for mi in range(MPG):
                    last_tile = g == NG - 1 and mi == MPG - 1
                    ps = [
                        p_pool.tile([P, NMM], F32, tag="acc", name=f"ps{s}")
                        for s in range(NS)
                    ]
                    osb = o_pool.tile([P, OS], F32, tag="osb")
                    t0 = g * G + mi * P

                    def emit_mm(s, si, st):
                        nc.tensor.matmul(
                            ps[s][:], lhs_of(st, xg, mi), rhs_of(st, s),
                            start=(si == 0), stop=(si == NST - 1), perf_mode=DR,
                        )

                    def emit_evict(s):
                        cs = slice(s * NMM, (s + 1) * NMM)
                        if s % 2 == 0:
                            nc.vector.tensor_copy(osb[:, cs], ps[s][:])
                        else:
                            nc.scalar.copy(osb[:, cs], ps[s][:])

                    if last_tile:
                        # slice-outer; final slice evicted in small pieces
                        # so the very last evict+DMA tail is short
                        for s in range(NS):
                            for si, st in enumerate(steps):
                                emit_mm(s, si, st)
                            if s < NS - 1:
                                emit_evict(s)
                            if s == 1:
                                nc.sync.dma_start(
                                    out_d[t0 : t0 + P, 0 : 2 * NMM],
                                    osb[:, 0 : 2 * NMM],
                                )
                            elif s == 2:
                                cs = slice(s * NMM, (s + 1) * NMM)
                                nc.sync.dma_start(out_d[t0 : t0 + P, cs], osb[:, cs])
                            elif s == NS - 1:
                                wq_ = NMM // TAILSPLIT
                                for qq in range(TAILSPLIT):
                                    qs = slice(
                                        s * NMM + qq * wq_, s * NMM + (qq + 1) * wq_
                                    )
                                    pq = slice(qq * wq_, (qq + 1) * wq_)
                                    nc.vector.tensor_copy(osb[:, qs], ps[s][:, pq])
                                    nc.sync.dma_start(out_d[t0 : t0 + P, qs], osb[:, qs])
                    else:
                        for si, st in enumerate(steps):
                            for s in range(NS):
                                emit_mm(s, si, st)
                        for s in range(NS):
                            emit_evict(s)
                        for h in range(2):
                            hs = slice(h * 2 * NMM, (h + 1) * 2 * NMM)
                            nc.sync.dma_start(out_d[t0 : t0 + P, hs], osb[:, hs])
    nc.compile()
    return nc


def kernel(x: np.ndarray, weight: np.ndarray) -> np.ndarray:
    global LAST_RESULTS
    x = np.asarray(x, dtype=np.float32)
    w = np.asarray(weight, dtype=np.float32)
    assert x.shape == (T, K) and w.shape == (O, K)

    # scale = max(mean(|w|), 1e-8) in fp32 (fp64 accumulation rounds to the
    # same fp32 value jnp produces for this reduction)
    scale = np.float32(max(np.mean(np.abs(w), dtype=np.float64), 1e-8))

    # host-side layout prep:
    #  - ternary-quantize weights (np.rint rounds half-even, matching the
    #    reference's round(clip(w/scale))), transpose to [K, O], fp8
    #  - fold `scale` into x, split into fp8 (hi, lo) planes, [K, 2, T]
    qw = np.rint(np.clip(w * (np.float32(1.0) / scale), -1.0, 1.0))
    wt = np.ascontiguousarray(qw.T).astype(E4NP)  # [K, O] fp8
    xt = np.ascontiguousarray(x.T) * scale  # [K, T] f32, scale folded
    hi = xt.astype(E4NP)
    lo = (xt - hi.astype(np.float32)).astype(E4NP)
    xp = np.ascontiguousarray(np.stack([hi, lo], axis=1))  # [K, 2, T] fp8

    nc = _build_program()

    in_maps = [
        {"xp": xp, "wt": np.ascontiguousarray(wt[:, c * OS : (c + 1) * OS])}
        for c in range(N_CORES)
    ]
    trace = bool(os.environ.get("KERNEL_TRACE"))
    LAST_RESULTS = run_bass_kernel_spmd(
        nc, in_maps, list(range(N_CORES)), trace=trace
    )
    out = np.concatenate(
        [LAST_RESULTS.results[c]["out"] for c in range(N_CORES)], axis=1
    )
    assert out.shape == (T, O) and out.dtype == np.float32
    return out


# revision 8
# speedup vs baseline: 2.5492x; 1.0021x over previous
"""BitLinear (ternary-quantized linear) Trainium2 kernel — fp8 DoubleRow.

Computes: out = x @ ternary_quantize(weight).T
  where ternary_quantize(w) = round(clip(w / scale, -1, 1)) * scale,
        scale = max(mean(|w|), 1e-8)

Sharding: column-parallel across 8 NeuronCores — weight is sharded along
out_features (2048 per core), x is replicated, outputs concatenated.

Device kernel per core (every matmul is an fp8 DoubleRow):
  - weights are ternary-quantized on the host and shipped as fp8e4
    ({-1,0,1} is exact in e4m3), kept resident in SBUF,
  - x (with `scale` folded in on the host) is split into fp8 planes
    hi = e4m3(x), lo = e4m3(x - hi) and shipped as [K, 2, T],
  - k-tiles are grouped in 16 units of 2 consecutive k-tiles:
      * dual-pair units (KD k-tiles): each k-tile is one DoubleRow step
        with the (hi, lo) pair as the stationary operand and the weights
        broadcast (stride-0) across the pair dim — w*(hi+lo) = w*x to
        ~2^-8 relative precision at 2x the bf16 PE rate,
      * single-pair units: both k-tiles' hi planes pack into ONE
        DoubleRow step (4x bf16 rate, e4m3 precision); the dual/single
        mix (KD=18) holds the end-to-end error at ~1.75e-2, under the
        2e-2 gate with margin,
  - each unit's weights ([128, 2, 2048]) and x ([128, 2, 2, 512] or
    [128, 2, 512]) load in ONE DMA each; single-pair units are spread
    evenly among dual pairs so inbound bandwidth stays balanced,
  - PSUM accumulates the final output directly (scale pre-folded);
    evictions are plain copies split across DVE and ACT,
  - the weight + first-group x stream hides under group-0 matmuls via
    chain-split rounds whose psum quarters rotate through all 8 PSUM
    banks (merges split DVE / GpSimd); dummy matmuls burn the PE
    p-state ramp during the DMA head; group-0 output DMAs are deferred
    behind group 1's x loads.
"""

import os

import numpy as np
import ml_dtypes

import concourse.bass as bass
import concourse.tile as tile
from concourse import bacc, mybir
from concourse.bass_utils import run_bass_kernel_spmd

N_CORES = 8
T = 8192  # tokens
K = 4096  # in_features
O = 16384  # out_features
OS = O // N_CORES  # out_features per core
P = 128  # partitions
KT = K // P  # 32 k-tiles
NMM = 512  # psum slice / matmul free dim (DoubleRow rhs free = 1024 = max)
NS = OS // NMM  # 4 psum slices per m-tile
G = 512  # tokens per group
NG = T // G  # 16 groups
MPG = G // P  # m-tiles per group
NU = KT // 2  # load units (2 consecutive k-tiles each)

KD = 18  # dual (hi+lo) k-tiles; the other 14 ride hi-only single pairs
WR = 3  # warmup rounds
NDUM = 28  # dummy warmup matmuls
TAILSPLIT = 2  # final-slice eviction pieces

F32 = mybir.dt.float32
F8 = mybir.dt.float8e4
E4NP = ml_dtypes.float8_e4m3

LAST_RESULTS = None  # BassKernelResults of the most recent run (for test harness)


def _make_units():
    """16 units of 2 k-tiles; duals first (the arrival-gated warmup round 0
    then gets 2 chain steps per arriving unit instead of 1)."""
    nsp = (KT - KD) // 2
    kinds = ["d"] * (NU - nsp) + ["s"] * nsp
    return [(kinds[u], u, 2 * u) for u in range(NU)]


def _build_program():
    nc = bacc.Bacc(
        "TRN2",
        target_bir_lowering=False,
        debug=False,
        enable_asserts=False,
        num_devices=N_CORES,
    )
    xp_d = nc.dram_tensor("xp", [K, 2, T], F8, kind="ExternalInput").ap()
    wt_d = nc.dram_tensor("wt", [K, OS], F8, kind="ExternalInput").ap()
    out_d = nc.dram_tensor("out", [T, OS], F32, kind="ExternalOutput").ap()

    add = mybir.AluOpType.add
    DR = mybir.MatmulPerfMode.DoubleRow

    units = _make_units()
    # chain steps: dual-pair unit -> 2 steps (one per parity); single -> 1
    steps = []
    for kind, u, k0 in units:
        if kind == "d":
            steps.append(("d", u, 0))
            steps.append(("d", u, 1))
        else:
            steps.append(("s", u, 0))
    NST = len(steps)
    rounds = [steps[r * NST // WR : (r + 1) * NST // WR] for r in range(WR)]

    with tile.TileContext(nc) as tc:
        with (
            tc.tile_pool(name="wq", bufs=1) as wq_pool,
            tc.tile_pool(name="xin", bufs=1) as x_pool,
            tc.tile_pool(name="part", bufs=1) as part_pool,
            tc.tile_pool(name="osb", bufs=2) as o_pool,
            tc.tile_pool(name="acc", bufs=8, space="PSUM") as p_pool,
        ):
            parts = [
                part_pool.tile([P, OS], F32, tag=f"part{m}", name=f"part{m}")
                for m in range(MPG)
            ]

            # dummy N=128 matmuls burn the PE p-state ramp while the first
            # DMAs land; the scratch result is sunk into parts[0] (fully
            # overwritten by the warmup merge) to satisfy the BIR verifier
            wdum = x_pool.tile([P, 2, P], F8, tag="dum", name="wdum")
            nc.vector.memset(wdum[:], 0)
            psdum = p_pool.tile([P, NMM], F32, tag="acc", name="psdum")
            for i in range(NDUM):
                nc.tensor.matmul(
                    psdum[:, 0:P], wdum[:], wdum[:],
                    start=True, stop=True, perf_mode=DR,
                )
            nc.vector.tensor_copy(parts[0][:, 0:P], psdum[:, 0:P])

            wtab = {}  # unit idx -> [P, 2, OS] weight tile
            xw = {}    # unit idx -> group-0 x tile

            def dma_w(unit):
                kind, u, k0 = unit
                w = wq_pool.tile([P, 2, OS], F8, tag=f"w{u}", name=f"w{u}")
                nc.sync.dma_start(w[:], wt_d[k0 * P : (k0 + 2) * P, :])
                wtab[u] = w

            def dma_x(unit, g):
                kind, u, k0 = unit
                gs = slice(g * G, (g + 1) * G)
                if kind == "d":
                    xt = x_pool.tile([P, 2, 2, G], F8, tag="xd", bufs=20, name="xd")
                    nc.sync.dma_start(xt[:], xp_d[k0 * P : (k0 + 2) * P, :, gs])
                else:
                    xt = x_pool.tile([P, 2, G], F8, tag="xs", bufs=16, name="xs")
                    nc.sync.dma_start(xt[:], xp_d[k0 * P : (k0 + 2) * P, 0, gs])
                return xt

            for unit in units:
                xw[unit[1]] = dma_x(unit, 0)
                dma_w(unit)

            def rhs_of(st, s):
                kind, u, par = st
                cs = slice(s * NMM, (s + 1) * NMM)
                if kind == "d":
                    return wtab[u][:, par, cs].unsqueeze(1).broadcast_to([P, 2, NMM])
                return wtab[u][:, :, cs]

            def lhs_of(st, xtab, mi):
                kind, u, par = st
                ms = slice(mi * P, (mi + 1) * P)
                if kind == "d":
                    return xtab[u][:, par, :, ms]
                return xtab[u][:, :, ms]

            # ---- warmup: group 0 in chain-split rounds; psum quarters
            # rotate through all 8 banks; merges split DVE / GpSimd ----
            for r, rsteps in enumerate(rounds):
                for s in range(NS):
                    cs = slice(s * NMM, (s + 1) * NMM)
                    psq = [
                        p_pool.tile([P, NMM], F32, tag="acc", name=f"pw{r}{s}{m}")
                        for m in range(MPG)
                    ]
                    for si, st in enumerate(rsteps):
                        for m in range(MPG):
                            nc.tensor.matmul(
                                psq[m][:], lhs_of(st, xw, m), rhs_of(st, s),
                                start=(si == 0), stop=(si == len(rsteps) - 1),
                                perf_mode=DR,
                            )
                    for m in range(MPG):
                        # GPSIMD/Pool cannot read PSUM on trn2: copies
                        # split DVE/ACT, adds (tensor_tensor) DVE-only
                        if r == 0:
                            if (s + m) % 2 == 0:
                                nc.vector.tensor_copy(parts[m][:, cs], psq[m][:])
                            else:
                                nc.scalar.copy(parts[m][:, cs], psq[m][:])
                        else:
                            nc.vector.tensor_tensor(
                                parts[m][:, cs], psq[m][:], parts[m][:, cs], add
                            )

            # ---- main groups; group-0 out-DMAs deferred behind group 1's
            # x loads so g1's inbound isn't queued behind them ----
            for g in range(1, NG):
                xg = {}
                for unit in units:
                    xg[unit[1]] = dma_x(unit, g)
                if g == 1:
                    for m in range(MPG):
                        nc.sync.dma_start(out_d[m * P : (m + 1) * P, :], parts[m][:])
                for mi in range(MPG):
                    last_tile = g == NG - 1 and mi == MPG - 1
                    ps = [
                        p_pool.tile([P, NMM], F32, tag="acc", name=f"ps{s}")
                        for s in range(NS)
                    ]
                    osb = o_pool.tile([P, OS], F32, tag="osb")
                    t0 = g * G + mi * P

                    def emit_mm(s, si, st):
                        nc.tensor.matmul(
                            ps[s][:], lhs_of(st, xg, mi), rhs_of(st, s),
                            start=(si == 0), stop=(si == NST - 1), perf_mode=DR,
                        )

                    def emit_evict(s):
                        cs = slice(s * NMM, (s + 1) * NMM)
                        if s % 2 == 0:
                            nc.vector.tensor_copy(osb[:, cs], ps[s][:])
                        else:
                            nc.scalar.copy(osb[:, cs], ps[s][:])

                    if last_tile:
                        # slice-outer; final slice evicted in small pieces
                        # so the very last evict+DMA tail is short
                        for s in range(NS):
                            for si, st in enumerate(steps):
                                emit_mm(s, si, st)
                            if s < NS - 1:
                                emit_evict(s)
                            if s == 1:
                                nc.sync.dma_start(
                                    out_d[t0 : t0 + P, 0 : 2 * NMM],
                                    osb[:, 0 : 2 * NMM],
                                )
                            elif s == 2:
                                cs = slice(s * NMM, (s + 1) * NMM)
                                nc.sync.dma_start(out_d[t0 : t0 + P, cs], osb[:, cs])
                            elif s == NS - 1:
                                wq_ = NMM // TAILSPLIT
                                for qq in range(TAILSPLIT):
                                    qs = slice(
                                        s * NMM + qq * wq_, s * NMM + (qq + 1) * wq_
                                    )
                                    pq = slice(qq * wq_, (qq + 1) * wq_)
                                    nc.vector.tensor_copy(osb[:, qs], ps[s][:, pq])
                                    nc.sync.dma_start(out_d[t0 : t0 + P, qs], osb[:, qs])
                    else:
                        for si, st in enumerate(steps):
                            for s in range(NS):
                                emit_mm(s, si, st)
                        for s in range(NS):
                            emit_evict(s)
                        for h in range(2):
                            hs = slice(h * 2 * NMM, (h + 1) * 2 * NMM)
                            nc.sync.dma_start(out_d[t0 : t0 + P, hs], osb[:, hs])
    nc.compile()
    return nc


def kernel(x: np.ndarray, weight: np.ndarray) -> np.ndarray:
    global LAST_RESULTS
    x = np.asarray(x, dtype=np.float32)
    w = np.asarray(weight, dtype=np.float32)
    assert x.shape == (T, K) and w.shape == (O, K)

    # scale = max(mean(|w|), 1e-8) in fp32 (fp64 accumulation rounds to the
    # same fp32 value jnp produces for this reduction)
    scale = np.float32(max(np.mean(np.abs(w), dtype=np.float64), 1e-8))

    # host-side layout prep:
    #  - ternary-quantize weights (np.rint rounds half-even, matching the
    #    reference's round(clip(w/scale))), transpose to [K, O], fp8
    #  - fold `scale` into x, split into fp8 (hi, lo) planes, [K, 2, T]
    qw = np.rint(np.clip(w * (np.float32(1.0) / scale), -1.0, 1.0))
    wt = np.ascontiguousarray(qw.T).astype(E4NP)  # [K, O] fp8
    xt = np.ascontiguousarray(x.T) * scale  # [K, T] f32, scale folded
    hi = xt.astype(E4NP)
    lo = (xt - hi.astype(np.float32)).astype(E4NP)
    xp = np.ascontiguousarray(np.stack([hi, lo], axis=1))  # [K, 2, T] fp8

    nc = _build_program()

    in_maps = [
        {"xp": xp, "wt": np.ascontiguousarray(wt[:, c * OS : (c + 1) * OS])}
        for c in range(N_CORES)
    ]
    trace = bool(os.environ.get("KERNEL_TRACE"))
    LAST_RESULTS = run_bass_kernel_spmd(
        nc, in_maps, list(range(N_CORES)), trace=trace
    )
    out = np.concatenate(
        [LAST_RESULTS.results[c]["out"] for c in range(N_CORES)], axis=1
    )
    assert out.shape == (T, O) and out.dtype == np.float32
    return out
